# revision 28
# baseline (speedup 1.0000x reference)
"""DifferentialCausalAttention on 8 Trainium2 NeuronCores.

Sharding: 8 cores = 2 batches x 4 head-groups (tensor-parallel over heads).
Core c handles batch b = c // 4 and head-group g = c % 4:
  - query heads 8g..8g+7 (4 pairs), kv heads 4g..4g+3, lambda cols 4g..4g+3
  - W_O rows 512g..512g+511 -> partial output, host-summed over the 4 groups.

v2 design (vs baseline):
  - Q^T/K^T/V/diffT stay SBUF-resident between phases (no DRAM round-trip).
  - RoPE: one ACT copy PSUM->SBUF, partition-rotation via SBUF-SBUF DMA on the
    gpsimd queue, then bf16 DVE mul/mul/add (2x mode).
  - Attention rowsum moved off the PE: DVE accumulates exp tiles into rs_acc
    (bf16), one ones-matmul per (head, superblock) reduces partitions.
  - Output projection (Wo) matmuls are interleaved into the attention loop as
    filler work per 512-wide L chunk, so PE bubbles from the S->exp->ctx
    dependency chain are filled and there is no separate phase-3 window.
  - Startup DMA ordering: first weight tile + first x chunk land before the
    bulk loads, so the first matmul issues within a few us.
"""
import os
from collections import deque
from contextlib import ExitStack

import ml_dtypes
import numpy as np

import concourse.bass as bass
import concourse.mybir as mybir
import concourse.tile as tile
from concourse import bacc
from concourse.bass_utils import run_bass_kernel_spmd

F32 = mybir.dt.float32
F32R = mybir.dt.float32r
BF16 = mybir.dt.bfloat16

B, L, D, NH = 2, 2048, 2048, 16
DH = D // NH            # 128
G = 4                   # head groups (cores per batch)
NKV = NH // G           # kv heads per core = 4
NQ = 2 * NKV            # query heads per core = 8
CQK = NQ * DH + NKV * DH  # 1536 projection cols (Q then K)
CT = CQK // 128         # 12 column tiles (0-7 Q heads, 8-11 K heads)
DC = D // 128           # 16 contraction chunks
LCH = L // 512          # 4 L-chunks
LT = L // 128           # 16 L-tiles / q-tiles
SB = LT // 2            # 8 superblocks
SCALE = 1.0 / float(np.sqrt(DH))
ROPE_BASE = 10000.0


def build_kernel() -> bacc.Bacc:
    nc = bacc.Bacc("TRN2", target_bir_lowering=False, debug=False)

    # pre-tiled on host so every DMA is contiguous per partition
    xP = nc.dram_tensor("xP", [LCH, 128, DC, 512], BF16, kind="ExternalInput")
    WqkP = nc.dram_tensor("WqkP", [CT, 128, DC, 128], BF16, kind="ExternalInput")
    WvP = nc.dram_tensor("WvP", [128, DC, NKV * DH], BF16, kind="ExternalInput")
    WlP = nc.dram_tensor("WlP", [128, DC, 128], BF16, kind="ExternalInput")
    blv = nc.dram_tensor("blv", [128, 1], F32, kind="ExternalInput")
    Wo = nc.dram_tensor("Wo", [NKV * DH, D], BF16, kind="ExternalInput")
    cosT = nc.dram_tensor("cosT", [DH, L], BF16, kind="ExternalInput")
    sinTs = nc.dram_tensor("sinTs", [DH, L], BF16, kind="ExternalInput")
    maskT = nc.dram_tensor("maskT", [128, 256], BF16, kind="ExternalInput")
    onesin = nc.dram_tensor("onesin", [128, 128], F32R, kind="ExternalInput")
    onesb = nc.dram_tensor("onesb", [128, 1], BF16, kind="ExternalInput")
    outT = nc.dram_tensor("outT", [D, L], BF16, kind="ExternalOutput")

    with ExitStack() as ctx:
        tc = ctx.enter_context(tile.TileContext(nc))

        persist = ctx.enter_context(tc.tile_pool(name="persist", bufs=1))

        # ---- persistent SBUF tensors ----
        qres = persist.tile([128, LT, NQ, 128], BF16)   # Q^T roped, (t, h, l)
        kres = persist.tile([128, NKV, L], BF16)        # K^T roped
        vres = persist.tile([128, LT, NKV, 128], BF16)  # V, l on partitions
        diffT = persist.tile([128, NKV, L], BF16)       # (ctx0-lam*ctx1)/rs ^T
        wo_sb = persist.tile([128, NKV, D], BF16)
        wv_sb = persist.tile([128, DC, NKV * DH], BF16)
        wl_sb = persist.tile([128, DC, 128], BF16)
        cos_sb = persist.tile([128, L], BF16)
        sin_sb = persist.tile([128, L], BF16)
        lamT = persist.tile([1, NKV, L], BF16)          # sigmoid(x@Wl+bl), partition 0
        bl_sb = persist.tile([128, 1], F32)
        mask_sb = persist.tile([128, 256], BF16)
        onesf_sb = persist.tile([128, 128], F32R)
        onesb_sb = persist.tile([128, 1], BF16)

        # ================= Phase 1: projections + RoPE =================
        with tc.tile_pool(name="ph1", bufs=1) as ph1, \
                tc.tile_pool(name="ps1", bufs=1, space="PSUM") as ps1:

            # startup-critical loads first, on the sync queue; first matmul
            # only needs wt0 + the first x chunks, so split the x load
            wt0 = ph1.tile([128, DC, 128], BF16, name="wt", tag="wt", bufs=3)
            nc.sync.dma_start(wt0[:], WqkP[0])
            xs0 = ph1.tile([128, DC, 512], BF16, name="xs", tag="xs", bufs=2)
            nc.sync.dma_start(xs0[:, 0:2, :], xP[0, :, 0:2, :])
            nc.sync.dma_start(xs0[:, 2:DC, :], xP[0, :, 2:DC, :])
            # bulk loads ride behind on the gpsimd queue
            nc.gpsimd.dma_start(cos_sb[:], cosT[:, :])
            nc.gpsimd.dma_start(sin_sb[:], sinTs[:, :])
            nc.gpsimd.dma_start(wv_sb[:], WvP[:])
            nc.gpsimd.dma_start(wl_sb[:], WlP[:])
            nc.gpsimd.dma_start(bl_sb[:], blv[:, :])
            nc.gpsimd.dma_start(mask_sb[:], maskT[:, :])
            nc.gpsimd.dma_start(onesf_sb[:], onesin[:, :])
            nc.gpsimd.dma_start(onesb_sb[:], onesb[:, :])
            nc.gpsimd.dma_start(wo_sb[:], Wo.rearrange("(p d) o -> d p o", d=128))

            xs = xs0
            for lch in range(LCH):
                ls = slice(lch * 512, (lch + 1) * 512)
                xs_next = None
                if lch + 1 < LCH:
                    xs_next = ph1.tile([128, DC, 512], BF16, name="xs", tag="xs", bufs=2)

                # --- Q^T / K^T column tiles + RoPE ---
                for ct in range(CT):
                    if lch == 0 and ct == 0:
                        wt = wt0
                    else:
                        wt = ph1.tile([128, DC, 128], BF16, name="wt", tag="wt", bufs=3)
                        nc.sync.dma_start(wt[:], WqkP[ct])
                    if ct == 3 and xs_next is not None:
                        # next-lch x prefetch, emitted here so the first weight
                        # tiles of this lch don't queue behind a 2MB transfer
                        nc.sync.dma_start(xs_next[:], xP[lch + 1])
                    qk_ps = ps1.tile([128, 512], F32, name="qk_ps", tag="mmq", bufs=4)
                    for dc in range(DC):
                        nc.tensor.matmul(
                            qk_ps[:], wt[:, dc, :], xs[:, dc, :],
                            start=(dc == 0), stop=(dc == DC - 1),
                        )
                    # RoPE: qr = qk*cos + rot(qk)*sin_signed
                    qf = ph1.tile([128, 512], BF16, name="qf", tag="qf", bufs=3)
                    nc.scalar.copy(qf[:], qk_ps[:])
                    rot = ph1.tile([128, 512], BF16, name="rot", tag="rot", bufs=3)
                    nc.gpsimd.dma_start(rot[0:64, :], qf[64:128, :])
                    nc.gpsimd.dma_start(rot[64:128, :], qf[0:64, :])
                    t1 = ph1.tile([128, 512], BF16, name="t1", tag="t1", bufs=2)
                    nc.vector.tensor_mul(t1[:], qf[:], cos_sb[:, ls])
                    t2 = ph1.tile([128, 512], BF16, name="t2", tag="t2", bufs=2)
                    nc.vector.tensor_mul(t2[:], rot[:], sin_sb[:, ls])
                    if ct < NQ:
                        dst = qres[:, lch * 4:(lch + 1) * 4, ct, :]
                        nc.vector.tensor_add(
                            dst,
                            t1.rearrange("p (t l) -> p t l", t=4),
                            t2.rearrange("p (t l) -> p t l", t=4),
                        )
                    else:
                        nc.vector.tensor_add(kres[:, ct - NQ, ls], t1[:], t2[:])

                # --- V tiles (l on partitions via x-as-stationary) ---
                for lt in range(4):
                    v_ps = ps1.tile([128, 512], F32, name="v_ps", tag="mmq", bufs=4)
                    for dc in range(DC):
                        nc.tensor.matmul(
                            v_ps[:], xs[:, dc, lt * 128:(lt + 1) * 128], wv_sb[:, dc, :],
                            start=(dc == 0), stop=(dc == DC - 1),
                        )
                    nc.scalar.copy(
                        vres[:, lch * 4 + lt, :, :].rearrange("p h d -> p (h d)"),
                        v_ps[:],
                    )

                # --- lambda ---
                lam_ps = ps1.tile([128, 512], F32, name="lam_ps", tag="mmq", bufs=4)
                for dc in range(DC):
                    nc.tensor.matmul(
                        lam_ps[:], wl_sb[:, dc, :], xs[:, dc, :],
                        start=(dc == 0), stop=(dc == DC - 1),
                    )
                lam4 = ph1.tile([NKV, 512], F32, name="lam4", tag="lam4", bufs=2)
                nc.scalar.activation(
                    lam4[:], lam_ps[0:NKV, :],
                    mybir.ActivationFunctionType.Sigmoid, bias=bl_sb[0:NKV, 0:1],
                )
                nc.gpsimd.dma_start(lamT[0:1, :, ls], lam4[:])
                xs = xs_next

        # ============ Phase 2+3: causal attention + output projection ============
        # Per (sb, p) unit: S^T = K^T q over k-chunks 0..2sb+1, exp on ACT,
        # rowsum accumulated on DVE into rs_acc, ctx matmuls accumulate in PSUM.
        # Norm chains and Wo-projection quartets are deferred into a filler
        # queue and emitted between S-matmul groups to fill PE bubbles.
        with tc.tile_pool(name="ph2", bufs=1) as ph2, \
                tc.tile_pool(name="ps2", bufs=1, space="PSUM") as ps2:

            fillers = deque()

            def emit_fillers(n):
                for _ in range(min(n, len(fillers))):
                    fillers.popleft()()

            def make_norm(p, sb, rs_ps, ctxc):
                qtA = 2 * sb

                def norm():
                    recip = ph2.tile([1, 512], F32, name="recip", tag="recip", bufs=2)
                    nc.vector.reciprocal_approx_fast(recip[:], rs_ps[0:1, :])
                    r4 = recip.rearrange("p (t h l) -> p t h l", t=2, h=2)
                    cs = ph2.tile([1, 2, 2, 128], F32R, name="cs", tag="cs", bufs=2)
                    nc.vector.tensor_copy(cs[:, :, 0, :], r4[:, :, 0, :])
                    nc.vector.tensor_mul(
                        cs[:, :, 1, :], r4[:, :, 1, :],
                        lamT[0:1, p, qtA * 128:(qtA + 2) * 128].rearrange(
                            "p (t l) -> p t l", t=2
                        ),
                    )
                    b_ps = ps2.tile([128, 512], F32, name="b_ps", tag="ob", bufs=2)
                    nc.tensor.matmul(
                        b_ps[:], onesf_sb[0:1, :],
                        cs.rearrange("p t h l -> p (t h l)"),
                        start=True, stop=True, skip_group_check=True,
                    )
                    u = ph2.tile([128, 2, 2, 128], BF16, name="u", tag="u", bufs=2)
                    nc.vector.tensor_mul(
                        u.rearrange("p t h l -> p (t h l)"), ctxc[:], b_ps[:]
                    )
                    nc.vector.tensor_sub(
                        diffT[:, p, sb * 256:(sb + 1) * 256].rearrange(
                            "p (t l) -> p t l", t=2
                        ),
                        u[:, :, 0, :], u[:, :, 1, :],
                    )
                return norm

            def make_oquartet(qch, ot):
                def oq():
                    o_ps = ps2.tile([128, 512], F32, name="o_ps", tag="ob", bufs=2)
                    for p in range(NKV):
                        nc.tensor.matmul(
                            o_ps[:],
                            wo_sb[:, p, ot * 128:(ot + 1) * 128],
                            diffT[:, p, qch * 512:(qch + 1) * 512],
                            start=(p == 0), stop=(p == NKV - 1),
                            skip_group_check=True,
                        )
                    o_sb = ph2.tile([128, 512], BF16, name="o_sb", tag="osb", bufs=4)
                    if qch == 0:
                        # the last-processed chunk: DVE is idle in the drain
                        # window while ACT would serialize the tail
                        nc.vector.tensor_copy(o_sb[:], o_ps[:])
                    else:
                        nc.scalar.copy(o_sb[:], o_ps[:])
                    nc.sync.dma_start(
                        outT[ot * 128:(ot + 1) * 128, qch * 512:(qch + 1) * 512],
                        o_sb[:],
                    )
                return oq

            # big superblocks first: deep PE pipelines from the start, and the
            # Wo filler work for each L chunk becomes available early; the
            # final (smallest) superblocks leave only a short tail
            for sb in range(SB - 1, -1, -1):
                qtA, qtB = 2 * sb, 2 * sb + 1
                for p in range(NKV):
                    ctx_ps = ps2.tile([128, 512], F32, name="ctx_ps", tag="ctx", bufs=2)
                    # two half-accumulators so each full group is ONE 1024-wide add
                    rs_acc = ph2.tile([128, 2, 512], BF16, name="rs_acc", tag="rsa", bufs=2)
                    pend = deque()

                    def emit_block(st):
                        e_sb, segs, first = st
                        for j, kc, off, wid in segs:
                            nc.tensor.matmul(
                                ctx_ps[:, off:off + wid], vres[:, kc, p, :],
                                e_sb[:, j, off:off + wid],
                                start=(kc == 0), stop=(kc == qtB),
                                skip_group_check=True,
                            )
                        full = (len(segs) == 2 and segs[0][3] == 512
                                and segs[1][3] == 512)
                        ef = e_sb.rearrange("p a b -> p (a b)")
                        rf = rs_acc.rearrange("p a b -> p (a b)")
                        if full:
                            if first:
                                nc.vector.tensor_copy(rf[:], ef[:])
                            else:
                                nc.vector.tensor_add(rf[:], rf[:], ef[:])
                        else:
                            for j, kc, off, wid in segs:
                                sl = (slice(None), j, slice(off, off + wid))
                                if first and j == 0:
                                    nc.vector.tensor_copy(rs_acc[sl], e_sb[sl])
                                elif first and j == 1:
                                    nc.vector.tensor_copy(rs_acc[sl], e_sb[sl])
                                else:
                                    nc.vector.tensor_add(
                                        rs_acc[sl], rs_acc[sl], e_sb[sl]
                                    )

                    kcs = list(range(qtB + 1))
                    groups = [kcs[i:i + 2] for i in range(0, len(kcs), 2)]
                    for gk, grp in enumerate(groups):
                        s_ps = ps2.tile([128, 2, 512], F32, name="s_ps", tag="s2", bufs=2)
                        segs = []
                        for j, kc in enumerate(grp):
                            off, wid = (256, 256) if kc == qtB else (0, 512)
                            rhs = (qres[:, qtA:qtA + 2, 2 * p:2 * p + 2, :]
                                   if wid == 512
                                   else qres[:, qtB, 2 * p:2 * p + 2, :])
                            nc.tensor.matmul(
                                s_ps[:, j, off:off + wid],
                                kres[:, p, kc * 128:(kc + 1) * 128],
                                rhs,
                                start=True, stop=True, skip_group_check=True,
                            )
                            segs.append((j, kc, off, wid))
                        emit_fillers(1)
                        while len(pend) >= 3:
                            emit_block(pend.popleft())
                        e_sb = ph2.tile([128, 2, 512], BF16, name="e_sb", tag="e", bufs=6)
                        if len(segs) == 2 and segs[0][3] == 512 and segs[1][3] == 512:
                            nc.scalar.activation(
                                e_sb.rearrange("p a b -> p (a b)"),
                                s_ps.rearrange("p a b -> p (a b)"),
                                mybir.ActivationFunctionType.Exp, scale=SCALE,
                            )
                        else:
                            for j, kc, off, wid in segs:
                                nc.scalar.activation(
                                    e_sb[:, j, off:off + wid], s_ps[:, j, off:off + wid],
                                    mybir.ActivationFunctionType.Exp, scale=SCALE,
                                )
                        for j, kc, off, wid in segs:
                            if kc == qtA:
                                nc.vector.tensor_mul(
                                    e_sb[:, j, 0:256], e_sb[:, j, 0:256], mask_sb[:]
                                )
                            elif kc == qtB:
                                nc.vector.tensor_mul(
                                    e_sb[:, j, 256:512], e_sb[:, j, 256:512], mask_sb[:]
                                )
                        pend.append((e_sb, segs, gk == 0))
                    for st in pend:
                        emit_block(st)
                    # rowsum partition-reduction: merge the two half-accumulators
                    # on DVE (PE is the binding engine here), then one matmul
                    w1 = 256 if sb == 0 else 512
                    nc.vector.tensor_add(
                        rs_acc[:, 0, 512 - w1:512], rs_acc[:, 0, 512 - w1:512],
                        rs_acc[:, 1, 512 - w1:512],
                    )
                    rs_ps = ps2.tile([128, 512], F32, name="rs_ps", tag="ob", bufs=2)
                    nc.tensor.matmul(
                        rs_ps[0:1, :], onesb_sb[:, 0:1], rs_acc[:, 0, :],
                        start=True, stop=True, skip_group_check=True,
                    )
                    ctxc = ph2.tile([128, 512], BF16, name="ctxc", tag="ctxc", bufs=2)
                    nc.vector.tensor_copy(ctxc[:], ctx_ps[:])
                    fillers.appendleft(make_norm(p, sb, rs_ps, ctxc))
                if sb % 2 == 0:
                    qch = sb // 2
                    for ot in range(LT):
                        fillers.append(make_oquartet(qch, ot))
            while fillers:
                fillers.popleft()()

    nc.finalize()
    return nc


def _host_tables():
    half = DH // 2
    inv_freq = 1.0 / (ROPE_BASE ** (np.arange(0, half, dtype=np.float64) * 2.0 / DH))
    freqs = np.arange(L, dtype=np.float64)[:, None] * inv_freq[None, :]  # [L, half]
    emb = np.concatenate([freqs, freqs], axis=-1)  # [L, DH]
    cosT = np.ascontiguousarray(np.cos(emb).T.astype(np.float32))  # [DH, L]
    sinT = np.sin(emb).T.astype(np.float32)
    sinTs = np.concatenate([-sinT[:half], sinT[half:]], axis=0)
    sinTs = np.ascontiguousarray(sinTs.astype(np.float32))
    tri = np.triu(np.ones((128, 128), dtype=np.float32))  # keep k' <= q'
    maskT = np.ascontiguousarray(np.concatenate([tri, tri], axis=1))
    ones = np.ones((128, 128), dtype=np.float32)
    return cosT, sinTs, maskT, ones


_NC_CACHE = []


def kernel(x, Wq, Wk, Wv, Wl, bl, Wo):
    bf16 = ml_dtypes.bfloat16
    x = np.asarray(x, dtype=np.float32)
    Wq = np.asarray(Wq, dtype=np.float32)
    Wk = np.asarray(Wk, dtype=np.float32)
    Wv = np.asarray(Wv, dtype=np.float32)
    Wl = np.asarray(Wl, dtype=np.float32)
    bl = np.asarray(bl, dtype=np.float32)
    Wo = np.asarray(Wo, dtype=np.float32)

    cosT, sinTs, maskT, ones = _host_tables()
    Wq3 = Wq.reshape(D, 2 * NH, DH)
    Wk3 = Wk.reshape(D, NH, DH)

    def tile_in(w, ncols):
        # [D, C] -> [C//128, 128, DC, 128]: per-partition-contiguous DMA layout
        t = w.reshape(DC, 128, ncols // 128, 128)
        return np.ascontiguousarray(t.transpose(2, 1, 0, 3)).astype(bf16)

    def tile_flat(w, ncols):
        # [D, C] -> [128, DC, C]: per-partition-contiguous, all cols together
        t = w.reshape(DC, 128, ncols)
        return np.ascontiguousarray(t.transpose(1, 0, 2)).astype(bf16)

    in_maps = []
    xPs = {}
    for b in range(B):
        t = x[b].T.reshape(DC, 128, LCH, 512)  # [dc, p, lch, l]
        xPs[b] = np.ascontiguousarray(t.transpose(2, 1, 0, 3)).astype(bf16)
    for c in range(8):
        b, g = divmod(c, G)
        wq_s = Wq3[:, 8 * g:8 * g + NQ, :].reshape(D, NQ * DH)
        wk_s = Wk3[:, G * g:G * g + NKV, :].reshape(D, NKV * DH)
        wv_s = Wv[:, DH * G * g:DH * G * g + NKV * DH]
        wl_s = np.pad(Wl[:, G * g:G * g + NKV], ((0, 0), (0, 128 - NKV)))
        in_maps.append({
            "xP": xPs[b],
            "WqkP": tile_in(np.concatenate([wq_s, wk_s], axis=1), CQK),
            "WvP": tile_flat(wv_s, NKV * DH),
            "WlP": tile_flat(wl_s, 128),
            "blv": np.ascontiguousarray(
                np.pad(bl[G * g:G * g + NKV], (0, 128 - NKV)).reshape(128, 1)),
            "Wo": np.ascontiguousarray(Wo[512 * g:512 * (g + 1), :]).astype(bf16),
            "cosT": cosT.astype(bf16),
            "sinTs": sinTs.astype(bf16),
            "maskT": maskT.astype(bf16),
            "onesin": ones,
            "onesb": np.ones((128, 1), dtype=np.float32).astype(bf16),
        })

    if not _NC_CACHE:
        _NC_CACHE.append(build_kernel())
    nc = _NC_CACHE[0]
    res = run_bass_kernel_spmd(nc, in_maps, core_ids=list(range(8)))

    out = np.empty((B, L, D), dtype=np.float32)
    for b in range(B):
        acc = res.results[4 * b]["outT"].astype(np.float32)
        for g in range(1, G):
            acc += res.results[4 * b + g]["outT"].astype(np.float32)
        out[b] = acc.T
    return out


# revision 38
# speedup vs baseline: 1.0013x; 1.0013x over previous
"""DifferentialCausalAttention on 8 Trainium2 NeuronCores.

Sharding: 8 cores = 2 batches x 4 head-groups (tensor-parallel over heads).
Core c handles batch b = c // 4 and head-group g = c % 4:
  - query heads 8g..8g+7 (4 pairs), kv heads 4g..4g+3, lambda cols 4g..4g+3
  - W_O rows 512g..512g+511 -> partial output, host-summed over the 4 groups.

v2 design (vs baseline):
  - Q^T/K^T/V/diffT stay SBUF-resident between phases (no DRAM round-trip).
  - RoPE: one ACT copy PSUM->SBUF, partition-rotation via SBUF-SBUF DMA on the
    gpsimd queue, then bf16 DVE mul/mul/add (2x mode).
  - Attention rowsum moved off the PE: DVE accumulates exp tiles into rs_acc
    (bf16), one ones-matmul per (head, superblock) reduces partitions.
  - Output projection (Wo) matmuls are interleaved into the attention loop as
    filler work per 512-wide L chunk, so PE bubbles from the S->exp->ctx
    dependency chain are filled and there is no separate phase-3 window.
  - Startup DMA ordering: first weight tile + first x chunk land before the
    bulk loads, so the first matmul issues within a few us.
"""
import os
from collections import deque
from contextlib import ExitStack

import ml_dtypes
import numpy as np

import concourse.bass as bass
import concourse.mybir as mybir
import concourse.tile as tile
from concourse import bacc
from concourse.bass_utils import run_bass_kernel_spmd

F32 = mybir.dt.float32
F32R = mybir.dt.float32r
BF16 = mybir.dt.bfloat16

B, L, D, NH = 2, 2048, 2048, 16
DH = D // NH            # 128
G = 4                   # head groups (cores per batch)
NKV = NH // G           # kv heads per core = 4
NQ = 2 * NKV            # query heads per core = 8
CQK = NQ * DH + NKV * DH  # 1536 projection cols (Q then K)
CT = CQK // 128         # 12 column tiles (0-7 Q heads, 8-11 K heads)
DC = D // 128           # 16 contraction chunks
LCH = L // 512          # 4 L-chunks
LT = L // 128           # 16 L-tiles / q-tiles
SB = LT // 2            # 8 superblocks
SCALE = 1.0 / float(np.sqrt(DH))
ROPE_BASE = 10000.0


def build_kernel() -> bacc.Bacc:
    nc = bacc.Bacc("TRN2", target_bir_lowering=False, debug=False)

    # pre-tiled on host so every DMA is contiguous per partition
    xP = nc.dram_tensor("xP", [LCH, 128, DC, 512], BF16, kind="ExternalInput")
    WqkP = nc.dram_tensor("WqkP", [CT, 128, DC, 128], BF16, kind="ExternalInput")
    WvP = nc.dram_tensor("WvP", [128, DC, NKV * DH], BF16, kind="ExternalInput")
    WlP = nc.dram_tensor("WlP", [128, DC, 128], BF16, kind="ExternalInput")
    blv = nc.dram_tensor("blv", [128, 1], F32, kind="ExternalInput")
    Wo = nc.dram_tensor("Wo", [NKV * DH, D], BF16, kind="ExternalInput")
    cosT = nc.dram_tensor("cosT", [DH, L], BF16, kind="ExternalInput")
    sinTs = nc.dram_tensor("sinTs", [DH, L], BF16, kind="ExternalInput")
    maskT = nc.dram_tensor("maskT", [128, 256], BF16, kind="ExternalInput")
    stepT = nc.dram_tensor("stepT", [128, 128], BF16, kind="ExternalInput")
    negII = nc.dram_tensor("negII", [128, 256], BF16, kind="ExternalInput")
    onesin = nc.dram_tensor("onesin", [128, 128], F32R, kind="ExternalInput")
    onesb = nc.dram_tensor("onesb", [128, 1], BF16, kind="ExternalInput")
    outT = nc.dram_tensor("outT", [D, L], BF16, kind="ExternalOutput")

    with ExitStack() as ctx:
        tc = ctx.enter_context(tile.TileContext(nc))

        persist = ctx.enter_context(tc.tile_pool(name="persist", bufs=1))

        # ---- persistent SBUF tensors ----
        qres = persist.tile([128, LT, NQ, 128], BF16)   # Q^T roped, (t, h, l)
        kres = persist.tile([128, NKV, L], BF16)        # K^T roped
        vres = persist.tile([128, LT, NKV, 128], BF16)  # V, l on partitions
        diffT = persist.tile([128, NKV, L], BF16)       # (ctx0-lam*ctx1)/rs ^T
        wo_sb = persist.tile([128, NKV, D], BF16)
        wv_sb = persist.tile([128, DC, NKV * DH], BF16)
        wl_sb = persist.tile([128, DC, 128], BF16)
        cos_sb = persist.tile([128, L], BF16)
        sin_sb = persist.tile([128, L], BF16)
        lamT = persist.tile([1, NKV, L], BF16)          # sigmoid(x@Wl+bl), partition 0
        bl_sb = persist.tile([128, 1], F32)
        mask_sb = persist.tile([128, 256], BF16)
        step_sb = persist.tile([128, 128], BF16)
        negii_sb = persist.tile([128, 256], BF16)
        onesf_sb = persist.tile([128, 128], F32R)
        onesb_sb = persist.tile([128, 1], BF16)

        # ================= Phase 1: projections + RoPE =================
        with tc.tile_pool(name="ph1", bufs=1) as ph1, \
                tc.tile_pool(name="ps1", bufs=1, space="PSUM") as ps1:

            # startup-critical loads first, on the sync queue; first matmul
            # only needs wt0 + the first x chunks, so split the x load
            wt0 = ph1.tile([128, DC, 128], BF16, name="wt", tag="wt", bufs=3)
            nc.sync.dma_start(wt0[:], WqkP[0])
            xs0 = ph1.tile([128, DC, 512], BF16, name="xs", tag="xs", bufs=2)
            nc.sync.dma_start(xs0[:, 0:2, :], xP[0, :, 0:2, :])
            nc.sync.dma_start(xs0[:, 2:DC, :], xP[0, :, 2:DC, :])
            # bulk loads ride behind on the gpsimd queue
            nc.gpsimd.dma_start(cos_sb[:], cosT[:, :])
            nc.gpsimd.dma_start(sin_sb[:], sinTs[:, :])
            nc.gpsimd.dma_start(wv_sb[:], WvP[:])
            nc.gpsimd.dma_start(wl_sb[:], WlP[:])
            nc.gpsimd.dma_start(bl_sb[:], blv[:, :])
            nc.gpsimd.dma_start(mask_sb[:], maskT[:, :])
            nc.gpsimd.dma_start(step_sb[:], stepT[:, :])
            nc.gpsimd.dma_start(negii_sb[:], negII[:, :])
            nc.gpsimd.dma_start(onesf_sb[:], onesin[:, :])
            nc.gpsimd.dma_start(onesb_sb[:], onesb[:, :])
            nc.gpsimd.dma_start(wo_sb[:], Wo.rearrange("(p d) o -> d p o", d=128))

            xs = xs0
            for lch in range(LCH):
                ls = slice(lch * 512, (lch + 1) * 512)
                xs_next = None
                if lch + 1 < LCH:
                    xs_next = ph1.tile([128, DC, 512], BF16, name="xs", tag="xs", bufs=2)

                # --- Q^T / K^T column tiles + RoPE ---
                for ct in range(CT):
                    if lch == 0 and ct == 0:
                        wt = wt0
                    else:
                        wt = ph1.tile([128, DC, 128], BF16, name="wt", tag="wt", bufs=3)
                        nc.sync.dma_start(wt[:], WqkP[ct])
                    if ct == 3 and xs_next is not None:
                        # next-lch x prefetch, emitted here so the first weight
                        # tiles of this lch don't queue behind a 2MB transfer
                        nc.sync.dma_start(xs_next[:], xP[lch + 1])
                    qk_ps = ps1.tile([128, 512], F32, name="qk_ps", tag="mmq", bufs=4)
                    for dc in range(DC):
                        nc.tensor.matmul(
                            qk_ps[:], wt[:, dc, :], xs[:, dc, :],
                            start=(dc == 0), stop=(dc == DC - 1),
                        )
                    # RoPE: qr = qk*cos + rot(qk)*sin_signed
                    qf = ph1.tile([128, 512], BF16, name="qf", tag="qf", bufs=3)
                    nc.scalar.copy(qf[:], qk_ps[:])
                    rot = ph1.tile([128, 512], BF16, name="rot", tag="rot", bufs=3)
                    nc.gpsimd.dma_start(rot[0:64, :], qf[64:128, :])
                    nc.gpsimd.dma_start(rot[64:128, :], qf[0:64, :])
                    t1 = ph1.tile([128, 512], BF16, name="t1", tag="t1", bufs=2)
                    nc.vector.tensor_mul(t1[:], qf[:], cos_sb[:, ls])
                    t2 = ph1.tile([128, 512], BF16, name="t2", tag="t2", bufs=2)
                    nc.vector.tensor_mul(t2[:], rot[:], sin_sb[:, ls])
                    if ct < NQ:
                        dst = qres[:, lch * 4:(lch + 1) * 4, ct, :]
                        nc.vector.tensor_add(
                            dst,
                            t1.rearrange("p (t l) -> p t l", t=4),
                            t2.rearrange("p (t l) -> p t l", t=4),
                        )
                    else:
                        nc.vector.tensor_add(kres[:, ct - NQ, ls], t1[:], t2[:])

                # --- V tiles (l on partitions via x-as-stationary) ---
                for lt in range(4):
                    v_ps = ps1.tile([128, 512], F32, name="v_ps", tag="mmq", bufs=4)
                    for dc in range(DC):
                        nc.tensor.matmul(
                            v_ps[:], xs[:, dc, lt * 128:(lt + 1) * 128], wv_sb[:, dc, :],
                            start=(dc == 0), stop=(dc == DC - 1),
                        )
                    nc.scalar.copy(
                        vres[:, lch * 4 + lt, :, :].rearrange("p h d -> p (h d)"),
                        v_ps[:],
                    )

                # --- lambda ---
                lam_ps = ps1.tile([128, 512], F32, name="lam_ps", tag="mmq", bufs=4)
                for dc in range(DC):
                    nc.tensor.matmul(
                        lam_ps[:], wl_sb[:, dc, :], xs[:, dc, :],
                        start=(dc == 0), stop=(dc == DC - 1),
                    )
                lam4 = ph1.tile([NKV, 512], F32, name="lam4", tag="lam4", bufs=2)
                nc.scalar.activation(
                    lam4[:], lam_ps[0:NKV, :],
                    mybir.ActivationFunctionType.Sigmoid, bias=bl_sb[0:NKV, 0:1],
                )
                nc.gpsimd.dma_start(lamT[0:1, :, ls], lam4[:])
                xs = xs_next

        # ============ Phase 2+3: causal attention + output projection ============
        # Per (sb, p) unit: S^T = K^T q over k-chunks 0..2sb+1, exp on ACT,
        # rowsum accumulated on DVE into rs_acc, ctx matmuls accumulate in PSUM.
        # Norm chains and Wo-projection quartets are deferred into a filler
        # queue and emitted between S-matmul groups to fill PE bubbles.
        with tc.tile_pool(name="ph2", bufs=1) as ph2, \
                tc.tile_pool(name="ps2", bufs=1, space="PSUM") as ps2:

            fillers = deque()

            def emit_fillers(n):
                for _ in range(min(n, len(fillers))):
                    fillers.popleft()()

            def make_norm(p, sb, rs_ps, ctxc):
                qtA = 2 * sb

                def norm():
                    recip = ph2.tile([1, 512], F32, name="recip", tag="recip", bufs=2)
                    nc.vector.reciprocal_approx_fast(recip[:], rs_ps[0:1, :])
                    r4 = recip.rearrange("p (t h l) -> p t h l", t=2, h=2)
                    cs = ph2.tile([1, 2, 2, 128], F32R, name="cs", tag="cs", bufs=2)
                    nc.vector.tensor_copy(cs[:, :, 0, :], r4[:, :, 0, :])
                    nc.vector.tensor_mul(
                        cs[:, :, 1, :], r4[:, :, 1, :],
                        lamT[0:1, p, qtA * 128:(qtA + 2) * 128].rearrange(
                            "p (t l) -> p t l", t=2
                        ),
                    )
                    b_ps = ps2.tile([128, 512], F32, name="b_ps", tag="ob", bufs=2)
                    nc.tensor.matmul(
                        b_ps[:], onesf_sb[0:1, :],
                        cs.rearrange("p t h l -> p (t h l)"),
                        start=True, stop=True, skip_group_check=True,
                    )
                    u = ph2.tile([128, 2, 2, 128], BF16, name="u", tag="u", bufs=2)
                    nc.vector.tensor_mul(
                        u.rearrange("p t h l -> p (t h l)"), ctxc[:], b_ps[:]
                    )
                    nc.vector.tensor_sub(
                        diffT[:, p, sb * 256:(sb + 1) * 256].rearrange(
                            "p (t l) -> p t l", t=2
                        ),
                        u[:, :, 0, :], u[:, :, 1, :],
                    )
                return norm

            def make_oquartet(qch, ot):
                def oq():
                    o_ps = ps2.tile([128, 512], F32, name="o_ps", tag="ob", bufs=2)
                    for p in range(NKV):
                        nc.tensor.matmul(
                            o_ps[:],
                            wo_sb[:, p, ot * 128:(ot + 1) * 128],
                            diffT[:, p, qch * 512:(qch + 1) * 512],
                            start=(p == 0), stop=(p == NKV - 1),
                            skip_group_check=True,
                        )
                    o_sb = ph2.tile([128, 512], BF16, name="o_sb", tag="osb", bufs=4)
                    nc.scalar.copy(o_sb[:], o_ps[:])
                    nc.sync.dma_start(
                        outT[ot * 128:(ot + 1) * 128, qch * 512:(qch + 1) * 512],
                        o_sb[:],
                    )
                return oq

            def make_ohalf(sbq, ot):
                # 256-wide Wo chunk for one superblock of the final L chunk,
                # so the sb=1 half interleaves into sb=0's units. DVE copies:
                # ACT would serialize the drain window.
                def oh():
                    o_ps = ps2.tile([128, 512], F32, name="o_ps", tag="ob", bufs=2)
                    for p in range(NKV):
                        nc.tensor.matmul(
                            o_ps[:, 0:256],
                            wo_sb[:, p, ot * 128:(ot + 1) * 128],
                            diffT[:, p, sbq * 256:(sbq + 1) * 256],
                            start=(p == 0), stop=(p == NKV - 1),
                            skip_group_check=True,
                        )
                    o_sb = ph2.tile([128, 512], BF16, name="o_sb", tag="osb", bufs=4)
                    nc.vector.tensor_copy(o_sb[:, 0:256], o_ps[:, 0:256])
                    nc.sync.dma_start(
                        outT[ot * 128:(ot + 1) * 128, sbq * 256:(sbq + 1) * 256],
                        o_sb[:, 0:256],
                    )
                return oh

            # big superblocks first: deep PE pipelines from the start, and the
            # Wo filler work for each L chunk becomes available early; the
            # final (smallest) superblocks leave only a short tail
            for sb in range(SB - 1, -1, -1):
                qtA, qtB = 2 * sb, 2 * sb + 1
                # small superblocks (processed last) are DVE-gated: shift the
                # per-unit fixed costs to PE/ACT, which idle there
                small = sb <= 3
                for p in range(NKV):
                    ctx_ps = ps2.tile([128, 512], F32, name="ctx_ps", tag="ctx", bufs=2)
                    # two half-accumulators so each full group is ONE 1024-wide add
                    rs_acc = ph2.tile([128, 2, 512], BF16, name="rs_acc", tag="rsa", bufs=2)
                    pend = deque()

                    def emit_block(st):
                        e_sb, segs, first = st
                        for j, kc, off, wid in segs:
                            nc.tensor.matmul(
                                ctx_ps[:, off:off + wid], vres[:, kc, p, :],
                                e_sb[:, j, off:off + wid],
                                start=(kc == 0), stop=(kc == qtB),
                                skip_group_check=True,
                            )
                        full = (len(segs) == 2 and segs[0][3] == 512
                                and segs[1][3] == 512)
                        ef = e_sb.rearrange("p a b -> p (a b)")
                        rf = rs_acc.rearrange("p a b -> p (a b)")
                        if full:
                            if first:
                                nc.vector.tensor_copy(rf[:], ef[:])
                            else:
                                nc.vector.tensor_add(rf[:], rf[:], ef[:])
                        else:
                            for j, kc, off, wid in segs:
                                sl = (slice(None), j, slice(off, off + wid))
                                if first and j == 0:
                                    nc.vector.tensor_copy(rs_acc[sl], e_sb[sl])
                                elif first and j == 1:
                                    nc.vector.tensor_copy(rs_acc[sl], e_sb[sl])
                                else:
                                    nc.vector.tensor_add(
                                        rs_acc[sl], rs_acc[sl], e_sb[sl]
                                    )

                    kcs = list(range(qtB + 1))
                    groups = [kcs[i:i + 2] for i in range(0, len(kcs), 2)]
                    for gk, grp in enumerate(groups):
                        s_ps = ps2.tile([128, 2, 512], F32, name="s_ps", tag="s2", bufs=2)
                        segs = []
                        for j, kc in enumerate(grp):
                            off, wid = (256, 256) if kc == qtB else (0, 512)
                            rhs = (qres[:, qtA:qtA + 2, 2 * p:2 * p + 2, :]
                                   if wid == 512
                                   else qres[:, qtB, 2 * p:2 * p + 2, :])
                            diag = kc in (qtA, qtB)
                            pe_mask = small and diag
                            nc.tensor.matmul(
                                s_ps[:, j, off:off + wid],
                                kres[:, p, kc * 128:(kc + 1) * 128],
                                rhs,
                                start=True, stop=not pe_mask,
                                skip_group_check=True,
                            )
                            if pe_mask:
                                # additive causal mask: -1e9 where k > q, so
                                # exp gives exact zeros and the DVE mask-mul
                                # is not needed
                                moff = 0 if kc == qtA else 256
                                nc.tensor.matmul(
                                    s_ps[:, j, moff:moff + 256],
                                    step_sb[:], negii_sb[:],
                                    start=False, stop=True, skip_group_check=True,
                                )
                            segs.append((j, kc, off, wid))
                        emit_fillers(1)
                        while len(pend) >= 3:
                            emit_block(pend.popleft())
                        e_sb = ph2.tile([128, 2, 512], BF16, name="e_sb", tag="e", bufs=6)
                        if len(segs) == 2 and segs[0][3] == 512 and segs[1][3] == 512:
                            nc.scalar.activation(
                                e_sb.rearrange("p a b -> p (a b)"),
                                s_ps.rearrange("p a b -> p (a b)"),
                                mybir.ActivationFunctionType.Exp, scale=SCALE,
                            )
                        else:
                            for j, kc, off, wid in segs:
                                nc.scalar.activation(
                                    e_sb[:, j, off:off + wid], s_ps[:, j, off:off + wid],
                                    mybir.ActivationFunctionType.Exp, scale=SCALE,
                                )
                        if not small:
                            for j, kc, off, wid in segs:
                                if kc == qtA:
                                    nc.vector.tensor_mul(
                                        e_sb[:, j, 0:256], e_sb[:, j, 0:256], mask_sb[:]
                                    )
                                elif kc == qtB:
                                    nc.vector.tensor_mul(
                                        e_sb[:, j, 256:512], e_sb[:, j, 256:512],
                                        mask_sb[:]
                                    )
                        pend.append((e_sb, segs, gk == 0))
                    for st in pend:
                        emit_block(st)
                    # rowsum partition-reduction over both half-accumulators:
                    # big sb -> merge halves on DVE, one matmul (PE-bound
                    # window); small sb -> two matmuls (DVE-bound window)
                    w1 = 256 if sb == 0 else 512
                    rs_ps = ps2.tile([128, 512], F32, name="rs_ps", tag="ob", bufs=2)
                    if small:
                        nc.tensor.matmul(
                            rs_ps[0:1, :], onesb_sb[:, 0:1], rs_acc[:, 0, :],
                            start=True, stop=False, skip_group_check=True,
                        )
                        nc.tensor.matmul(
                            rs_ps[0:1, 512 - w1:512], onesb_sb[:, 0:1],
                            rs_acc[:, 1, 512 - w1:512],
                            start=False, stop=True, skip_group_check=True,
                        )
                    else:
                        nc.vector.tensor_add(
                            rs_acc[:, 0, :], rs_acc[:, 0, :], rs_acc[:, 1, :]
                        )
                        nc.tensor.matmul(
                            rs_ps[0:1, :], onesb_sb[:, 0:1], rs_acc[:, 0, :],
                            start=True, stop=True, skip_group_check=True,
                        )
                    ctxc = ph2.tile([128, 512], BF16, name="ctxc", tag="ctxc", bufs=2)
                    if small:
                        nc.scalar.copy(ctxc[:], ctx_ps[:])
                    else:
                        nc.vector.tensor_copy(ctxc[:], ctx_ps[:])
                    fillers.appendleft(make_norm(p, sb, rs_ps, ctxc))
                if sb % 2 == 0 and sb > 0:
                    qch = sb // 2
                    for ot in range(LT):
                        fillers.append(make_oquartet(qch, ot))
                elif sb <= 1:
                    for ot in range(LT):
                        fillers.append(make_ohalf(sb, ot))
            while fillers:
                fillers.popleft()()

    nc.finalize()
    return nc


def _host_tables():
    half = DH // 2
    inv_freq = 1.0 / (ROPE_BASE ** (np.arange(0, half, dtype=np.float64) * 2.0 / DH))
    freqs = np.arange(L, dtype=np.float64)[:, None] * inv_freq[None, :]  # [L, half]
    emb = np.concatenate([freqs, freqs], axis=-1)  # [L, DH]
    cosT = np.ascontiguousarray(np.cos(emb).T.astype(np.float32))  # [DH, L]
    sinT = np.sin(emb).T.astype(np.float32)
    sinTs = np.concatenate([-sinT[:half], sinT[half:]], axis=0)
    sinTs = np.ascontiguousarray(sinTs.astype(np.float32))
    tri = np.triu(np.ones((128, 128), dtype=np.float32))  # keep k' <= q'
    maskT = np.ascontiguousarray(np.concatenate([tri, tri], axis=1))
    ones = np.ones((128, 128), dtype=np.float32)
    return cosT, sinTs, maskT, ones


_NC_CACHE = []


def kernel(x, Wq, Wk, Wv, Wl, bl, Wo):
    bf16 = ml_dtypes.bfloat16
    x = np.asarray(x, dtype=np.float32)
    Wq = np.asarray(Wq, dtype=np.float32)
    Wk = np.asarray(Wk, dtype=np.float32)
    Wv = np.asarray(Wv, dtype=np.float32)
    Wl = np.asarray(Wl, dtype=np.float32)
    bl = np.asarray(bl, dtype=np.float32)
    Wo = np.asarray(Wo, dtype=np.float32)

    cosT, sinTs, maskT, ones = _host_tables()
    Wq3 = Wq.reshape(D, 2 * NH, DH)
    Wk3 = Wk.reshape(D, NH, DH)

    def tile_in(w, ncols):
        # [D, C] -> [C//128, 128, DC, 128]: per-partition-contiguous DMA layout
        t = w.reshape(DC, 128, ncols // 128, 128)
        return np.ascontiguousarray(t.transpose(2, 1, 0, 3)).astype(bf16)

    def tile_flat(w, ncols):
        # [D, C] -> [128, DC, C]: per-partition-contiguous, all cols together
        t = w.reshape(DC, 128, ncols)
        return np.ascontiguousarray(t.transpose(1, 0, 2)).astype(bf16)

    in_maps = []
    xPs = {}
    for b in range(B):
        t = x[b].T.reshape(DC, 128, LCH, 512)  # [dc, p, lch, l]
        xPs[b] = np.ascontiguousarray(t.transpose(2, 1, 0, 3)).astype(bf16)
    for c in range(8):
        b, g = divmod(c, G)
        wq_s = Wq3[:, 8 * g:8 * g + NQ, :].reshape(D, NQ * DH)
        wk_s = Wk3[:, G * g:G * g + NKV, :].reshape(D, NKV * DH)
        wv_s = Wv[:, DH * G * g:DH * G * g + NKV * DH]
        wl_s = np.pad(Wl[:, G * g:G * g + NKV], ((0, 0), (0, 128 - NKV)))
        in_maps.append({
            "xP": xPs[b],
            "WqkP": tile_in(np.concatenate([wq_s, wk_s], axis=1), CQK),
            "WvP": tile_flat(wv_s, NKV * DH),
            "WlP": tile_flat(wl_s, 128),
            "blv": np.ascontiguousarray(
                np.pad(bl[G * g:G * g + NKV], (0, 128 - NKV)).reshape(128, 1)),
            "Wo": np.ascontiguousarray(Wo[512 * g:512 * (g + 1), :]).astype(bf16),
            "cosT": cosT.astype(bf16),
            "sinTs": sinTs.astype(bf16),
            "maskT": maskT.astype(bf16),
            "stepT": np.ascontiguousarray(
                np.triu(np.ones((128, 128), dtype=np.float32), k=1)).astype(bf16),
            "negII": np.ascontiguousarray(np.concatenate(
                [np.eye(128, dtype=np.float32)] * 2, axis=1) * -1e9).astype(bf16),
            "onesin": ones,
            "onesb": np.ones((128, 1), dtype=np.float32).astype(bf16),
        })

    if not _NC_CACHE:
        _NC_CACHE.append(build_kernel())
    nc = _NC_CACHE[0]
    res = run_bass_kernel_spmd(nc, in_maps, core_ids=list(range(8)))

    out = np.empty((B, L, D), dtype=np.float32)
    for b in range(B):
        acc = res.results[4 * b]["outT"].astype(np.float32)
        for g in range(1, G):
            acc += res.results[4 * b + g]["outT"].astype(np.float32)
        out[b] = acc.T
    return out
